# revision 43
# baseline (speedup 1.0000x reference)
"""AttentionBlock (GroupNorm + 4-head self-attention + proj + residual) on 8 trn2 cores.

Sharding: data-parallel over batch (B=16 -> 2 per core). Each core runs the full
block on its 2 batch elements; no collectives.

Device pipeline per batch (all layouts chosen so no on-device transposes are needed):
  - GroupNorm stats via bn_stats/bn_aggr + tiny PE matmuls for the cross-partition
    group combine (gamma/beta folded into the qkv weights on host); inv_std via a
    DVE bit-trick rsqrt + Newton so the ACT engine never leaves the Exp table.
  - h cast to fp8 (e4m3); qkv / V^T / proj GEMMs run fp8 DoubleRow (2 MAC/cell/cyc)
    with weights pre-scaled x8 on host for e4m3 range, compensated in the PSUM evac.
  - Q,K evac to f32r, so the S^T matmuls are full-precision (contraction is only
    d=64 there, DoubleRow cannot help them).
  - Scores computed transposed: S^T[m,n] = K^T Q; exp(S-3) on ACT straight out of
    PSUM into fp8 pair tiles (uniform e^-3 scale cancels in the normalization).
  - AV as fp8 DoubleRow over m-chunk pairs; the stationary [V_h | ones] blocks also
    emit the softmax denominator Z replicated across partitions for free.
  - Normalization reads O and Z directly from PSUM (reciprocal + mul), Z moved
    across partitions by a small DMA; output quantized to fp8 pair tiles for proj.
  - proj fp8 DoubleRow + residual from x with proj bias pre-added (no bias matmuls).
  - Emission interleaves batch-1 GEMMs and proj work as fillers inside batch-0's
    attention steps so the PE queue never stalls on the ACT exp stream.
"""

import numpy as np
from contextlib import ExitStack

import concourse.bass as bass
import concourse.bacc as bacc
import concourse.tile as tile
import concourse.mybir as mybir
from concourse.bass_utils import run_bass_kernel_spmd

F32 = mybir.dt.float32
F32R = mybir.dt.float32r
BF16 = mybir.dt.bfloat16
FP8 = mybir.dt.float8e4
I32 = mybir.dt.int32
DR = mybir.MatmulPerfMode.DoubleRow

B, C, HH, WW = 16, 256, 32, 32
N = HH * WW           # 1024 spatial positions
NH = 4                # heads
D = C // NH           # 64 head dim
G = 32                # groups
EPS = 1e-5
NCORES = 8
BL = B // NCORES      # batches per core

RSQRT_MAGIC = 0x5F3759DF
BITRSQRT = True
DEBUG = False


def build_bass():
    nc = bacc.Bacc("TRN2", target_bir_lowering=False, debug=False)

    x_d = nc.dram_tensor("x", [BL, C, N], F32, kind="ExternalInput").ap()
    wqk_d = nc.dram_tensor("wqk8", [128, 1024], F32, kind="ExternalInput").ap()
    wv_d = nc.dram_tensor("wv8", [128, 512], F32, kind="ExternalInput").ap()
    wp_d = nc.dram_tensor("wp8", [128, 512], F32, kind="ExternalInput").ap()
    bqk_d = nc.dram_tensor("bqk", [4, 128], F32, kind="ExternalInput").ap()
    bv_d = nc.dram_tensor("bv", [1, 256], F32R, kind="ExternalInput").ap()
    bp_d = nc.dram_tensor("bp2", [2, 128], F32, kind="ExternalInput").ap()
    gmap_d = nc.dram_tensor("gmap", [128, 16], F32, kind="ExternalInput").ap()
    gexp_d = nc.dram_tensor("gexp", [16, 128], F32, kind="ExternalInput").ap()
    y_d = nc.dram_tensor("y", [BL, C, N], F32, kind="ExternalOutput").ap()
    if DEBUG:
        dbg_st_d = nc.dram_tensor("dbg_st", [BL, 128, 4], F32, kind="ExternalOutput").ap()
        dbg_qk_d = nc.dram_tensor("dbg_qk", [4, 128, 1024], F32, kind="ExternalOutput").ap()
        dbg_h_d = nc.dram_tensor("dbg_h", [128, 2048], F32, kind="ExternalOutput").ap()
        dbg_vt_d = nc.dram_tensor("dbg_vt", [128, 1024], F32, kind="ExternalOutput").ap()
        dbg_o_d = nc.dram_tensor("dbg_o", [128, 2048], F32, kind="ExternalOutput").ap()
        dbg_po_d = nc.dram_tensor("dbg_po", [2, 128, 512], F32, kind="ExternalOutput").ap()
        dbg_rz_d = nc.dram_tensor("dbg_rz", [2, 128, 512], F32, kind="ExternalOutput").ap()

    Exp = mybir.ActivationFunctionType.Exp
    mult = mybir.AluOpType.mult
    sub = mybir.AluOpType.subtract
    add = mybir.AluOpType.add
    asr = mybir.AluOpType.arith_shift_right

    with tile.TileContext(nc) as tc, ExitStack() as ctx:
        consts = ctx.enter_context(tc.tile_pool(name="consts", bufs=1))
        xpool = ctx.enter_context(tc.tile_pool(name="xp", bufs=1))
        xbpool = ctx.enter_context(tc.tile_pool(name="xbp", bufs=1))
        hpool = ctx.enter_context(tc.tile_pool(name="hp", bufs=1))
        qkpool = ctx.enter_context(tc.tile_pool(name="qkp", bufs=1))
        vtpool = ctx.enter_context(tc.tile_pool(name="vtp", bufs=1))
        opool = ctx.enter_context(tc.tile_pool(name="op", bufs=1))
        gnpool = ctx.enter_context(tc.tile_pool(name="gnp", bufs=2))
        expool = ctx.enter_context(tc.tile_pool(name="exp", bufs=3))
        rzpool = ctx.enter_context(tc.tile_pool(name="rzp", bufs=3))
        outpool = ctx.enter_context(tc.tile_pool(name="outp", bufs=4))
        # PSUM: 8 banks x 2KB/partition.  ps_s: S^T tiles [128,1024] x2 = 4
        # banks; ps_o: AV accumulators [128,512] x2 = 2 banks; ps_f: filler/
        # GEMM psums [128,512] x2 = 2 banks.
        ps_s = ctx.enter_context(tc.tile_pool(name="pss", bufs=2, space="PSUM"))
        ps_o = ctx.enter_context(tc.tile_pool(name="pso", bufs=2, space="PSUM"))
        ps_f = ctx.enter_context(tc.tile_pool(name="psf", bufs=2, space="PSUM"))

        x_sb = [[None] * 2 for _ in range(BL)]
        xb_sb = [[None] * 2 for _ in range(BL)]
        h_sb = [None] * BL            # fp8 ct-pair tiles [128, 2*1024]
        qk_sb = [[None] * 4 for _ in range(BL)]   # f32r [128,1024]: q01 q23 k01 k23
        vt_sb = [[None] * 4 for _ in range(BL)]   # fp8 m-chunk pair tiles [128, 2*512]
        o_sb = [None] * BL            # fp8 p-pair tiles [128, 2*1024]

        # ---------------- DMAs: x b0 FIRST (GN gates everything), then the
        # tiny consts (pbv/warmups need them early), then weights, then x b1.
        for b in range(BL):
            for ct in range(2):
                xt = xpool.tile([128, N], F32, tag=f"x{b}{ct}", name=f"x{b}{ct}")
                x_sb[b][ct] = xt
                if b == 0:
                    nc.sync.dma_start(xt[:, 0:512], x_d[b, ct * 128:(ct + 1) * 128, 0:512])
                    nc.sync.dma_start(xt[:, 512:1024], x_d[b, ct * 128:(ct + 1) * 128, 512:1024])
        gmap_sb = consts.tile([128, 16], F32, tag="gmap")
        nc.sync.dma_start(gmap_sb[:], gmap_d[:])
        gexp_sb = consts.tile([16, 128], F32, tag="gexp")
        nc.sync.dma_start(gexp_sb[:], gexp_d[:])
        bqk_sb = consts.tile([128, 4], F32, tag="bqk")
        nc.sync.dma_start(bqk_sb[:], bqk_d.transpose([1, 0]))
        bp_sb = consts.tile([128, 2], F32, tag="bp")
        nc.sync.dma_start(bp_sb[:], bp_d.transpose([1, 0]))
        bv_sb = consts.tile([1, 256], F32R, tag="bv")
        nc.sync.dma_start(bv_sb[:], bv_d[:])
        wqk_f = consts.tile([128, 1024], F32, tag="wqk_f")
        nc.sync.dma_start(wqk_f[:, 0:512], wqk_d[:, 0:512])
        nc.sync.dma_start(wqk_f[:, 512:1024], wqk_d[:, 512:1024])
        wv_f = consts.tile([128, 512], F32, tag="wv_f")
        nc.sync.dma_start(wv_f[:], wv_d[:])
        wp_f = consts.tile([128, 512], F32, tag="wp_f")
        nc.sync.dma_start(wp_f[:], wp_d[:])
        for b in range(1, BL):
            for ct in range(2):
                xt = x_sb[b][ct]
                nc.sync.dma_start(xt[:, 0:512], x_d[b, ct * 128:(ct + 1) * 128, 0:512])
                nc.sync.dma_start(xt[:, 512:1024], x_d[b, ct * 128:(ct + 1) * 128, 512:1024])

        # ---------------- consts / fp8 weight casts ------------------------
        ones_f32 = consts.tile([128, 512], F32, tag="ones_f32")
        nc.vector.memset(ones_f32[:], 1.0)
        neg3 = consts.tile([128, 1], F32, tag="neg3")
        nc.vector.memset(neg3[:], -3.0)
        ones_sb = consts.tile([128, 512], F32R, tag="ones")
        nc.vector.tensor_copy(ones_sb[:, 0:512], ones_f32[:])
        # fp8 weight casts on the (otherwise idle until first exp) ACT engine,
        # keeping the DVE free for the GroupNorm stats/chain critical path.
        wqk8 = consts.tile([128, 1024], FP8, tag="wqk8")
        nc.scalar.copy(wqk8[:], wqk_f[:])
        wv8 = consts.tile([128, 512], FP8, tag="wv8")
        nc.scalar.copy(wv8[:], wv_f[:])
        wp8 = consts.tile([128, 512], FP8, tag="wp8")
        nc.scalar.copy(wp8[:], wp_f[:])
        wqk2 = wqk8[:].rearrange("p (s o) -> p s o", s=2)
        wv2 = wv8[:].rearrange("p (s o) -> p s o", s=2)
        wp2 = wp8[:].rearrange("p (s o) -> p s o", s=2)

        # bv broadcast to all partitions via a rank-1 matmul (done once)
        pbv = ps_f.tile([128, 256], F32, tag="f", name="pbv")
        nc.tensor.matmul(pbv[:], lhsT=ones_sb[0:1, 0:128], rhs=bv_sb[0:1, :],
                         start=True, stop=True)
        bvb = consts.tile([128, 256], F32, tag="bvb")
        nc.scalar.copy(bvb[:], pbv[:])

        def emit_warmups(n):
            # PE warm-up: the PE clock needs ~3us of sustained activity to
            # reach full p-state; run these in PE-queue slots where the PE
            # would otherwise wait on the DVE GroupNorm chain.
            for w in range(n):
                pw = ps_f.tile([128, 512], F32, tag="f", name="pw")
                nc.tensor.matmul(pw[:], lhsT=ones_sb[0:1, 0:128],
                                 rhs=ones_sb[0:1, :], start=True, stop=True)

        # ---------------- GroupNorm (per batch) ----------------------------
        bn6 = gnpool.tile([128, 96], F32, tag="bn6")
        mva = gnpool.tile([128, 8], F32, tag="mva")
        st_sb = [None] * BL  # [128, 4]: (s, t) x 2ct

        def emit_gn_stats(b):
            # stats per quarter so they trail the quarter-granular x DMAs
            for ct in range(2):
                u4 = 2 * b + ct
                xt = x_sb[b][ct]
                for q in range(4):
                    nc.vector.bn_stats(bn6[:, 24 * u4 + 6 * q:24 * u4 + 6 * q + 6],
                                       xt[:, q * 256:(q + 1) * 256])
                nc.vector.bn_aggr(mva[:, 2 * u4:2 * u4 + 2],
                                  bn6[:, 24 * u4:24 * u4 + 24])

        def emit_gn_chain(b):
            # per-group combine for batch b's 2 ct tiles -> s = rsqrt(var+eps),
            # t = mean*s, broadcast back to 128 partitions via gexp matmul.
            # Kept deliberately short: every op here is serial latency on the
            # critical path to the first qkv GEMM.
            mv = mva[:, 4 * b:4 * b + 4]
            mv2 = mv.rearrange("p (u c) -> p u c", c=2)
            m2a = gnpool.tile([128, 2], F32, tag=f"m2a{b}", name=f"m2a{b}")
            nc.vector.tensor_mul(m2a[:], mv2[:, :, 0], mv2[:, :, 0])
            psg = ps_f.tile([16, 6], F32, tag="f", name="psg")
            nc.tensor.matmul(psg[:, 0:4], lhsT=gmap_sb[:], rhs=mv, start=True,
                             stop=True, skip_group_check=True)
            nc.tensor.matmul(psg[:, 4:6], lhsT=gmap_sb[:], rhs=m2a[:], start=True,
                             stop=True, skip_group_check=True)
            g = gnpool.tile([16, 10], F32, tag=f"g{b}", name=f"g{b}")
            nc.vector.tensor_copy(g[:, 0:6], psg[:])
            g2 = g[:, 0:4].rearrange("p (u c) -> p u c", c=2)
            # var_g + eps = (E[var] + eps) + E[mean^2] - mean_g^2
            nc.vector.scalar_tensor_tensor(g[:, 6:8], g2[:, :, 1], EPS,
                                           g[:, 4:6], add, add)
            nc.vector.tensor_mul(g[:, 8:10], g2[:, :, 0], g2[:, :, 0])
            nc.vector.tensor_sub(g[:, 6:8], g[:, 6:8], g[:, 8:10])
            # rsqrt via bit trick + one Newton iteration (all DVE, no ACT —
            # the ACT engine then never leaves the Exp table)
            gi = g[:].bitcast(I32)
            nc.vector.tensor_scalar(gi[:, 8:10], gi[:, 6:8], 1, None, asr)
            nc.vector.tensor_scalar(gi[:, 8:10], gi[:, 8:10], -1, RSQRT_MAGIC,
                                    mult, add)
            sg = gnpool.tile([16, 4], F32, tag=f"sg{b}", name=f"sg{b}")
            s2 = sg[:].rearrange("p (u c) -> p u c", c=2)
            nc.vector.tensor_mul(g[:, 4:6], g[:, 8:10], g[:, 8:10])
            nc.vector.tensor_mul(g[:, 4:6], g[:, 4:6], g[:, 6:8])
            nc.vector.tensor_scalar(g[:, 4:6], g[:, 4:6], -0.5, 1.5, mult, add)
            nc.vector.tensor_mul(s2[:, :, 0], g[:, 8:10], g[:, 4:6])
            nc.vector.tensor_mul(s2[:, :, 1], g2[:, :, 0], s2[:, :, 0])
            psc = ps_f.tile([128, 4], F32, tag="f", name="psc")
            nc.tensor.matmul(psc[:], lhsT=gexp_sb[:], rhs=sg[:], start=True, stop=True)
            st = gnpool.tile([128, 4], F32, tag=f"st{b}", name=f"st{b}")
            st_sb[b] = st
            nc.vector.tensor_copy(st[:], psc[:])

        def emit_h(b):
            ht = hpool.tile([128, 2 * N], FP8, tag=f"h{b}", name=f"h{b}")
            h_sb[b] = ht
            st = st_sb[b]
            for ct in range(2):
                nc.vector.tensor_scalar(ht[:, N * ct:N * ct + N], x_sb[b][ct][:],
                                        st[:, 2 * ct:2 * ct + 1],
                                        st[:, 2 * ct + 1:2 * ct + 2], mult, sub)

        def emit_xb(b):
            for ct in range(2):
                xbt = xbpool.tile([128, N], F32, tag=f"xb{b}{ct}", name=f"xb{b}{ct}")
                xb_sb[b][ct] = xbt
                nc.vector.tensor_scalar(xbt[:], x_sb[b][ct][:],
                                        bp_sb[:, ct:ct + 1], None, add)

        # ---------------- batch GEMMs (fp8 DoubleRow) -----------------------
        # emit_qkv ot order: k01, k01(nch1) first so S can start early.
        QKV_SCALE = 0.125            # weights stored x8
        K_SCALE = 0.125 * 0.125      # extra d^-0.5 fold for K

        for b in range(BL):
            for ot in range(4):
                qk_sb[b][ot] = qkpool.tile([128, N], F32R, tag=f"qk{b}{ot}",
                                           name=f"qk{b}{ot}")

        def emit_qkv1(b, ot, nch):
            # one DR matmul: out psum [128,512] = (8W)^T h for ot block
            ns = slice(nch * 512, (nch + 1) * 512)
            h2 = h_sb[b][:].rearrange("p (s n) -> p s n", s=2)
            pq = ps_f.tile([128, 512], F32, tag="f", name="pq")
            nc.tensor.matmul(pq[:], lhsT=wqk2[:, :, ot * 128:(ot + 1) * 128],
                             rhs=h2[:, :, ns], start=True, stop=True, perf_mode=DR)
            sc = K_SCALE if ot >= 2 else QKV_SCALE
            nc.vector.tensor_scalar(qk_sb[b][ot][:, ns], pq[:], sc,
                                    bqk_sb[:, ot:ot + 1], mult, add)

        def emit_vt1(b, m):
            # V^T chunk m: psum [128,256] = h[:,mc]^T (8Wv); build fp8 pair tile
            j, slot = divmod(m, 2)
            if slot == 0:
                vt_sb[b][j] = vtpool.tile([128, 1024], FP8, tag=f"vt{b}{j}",
                                          name=f"vt{b}{j}")
            mc = slice(m * 128, (m + 1) * 128)
            h2 = h_sb[b][:].rearrange("p (s n) -> p s n", s=2)
            pv = ps_f.tile([128, 256], F32, tag="f", name="pv")
            nc.tensor.matmul(pv[:], lhsT=h2[:, :, mc], rhs=wv2[:],
                             start=True, stop=True, perf_mode=DR)
            vt = vt_sb[b][j][:, 512 * slot:512 * slot + 512]
            vt4 = vt.rearrange("p (a u v d) -> p a u v d", a=2, u=2, v=2)
            pv4 = pv[:].rearrange("p (a w d) -> p a w d", a=2, w=2)
            bvb4 = bvb[:].rearrange("p (a w d) -> p a w d", a=2, w=2)
            nc.vector.scalar_tensor_tensor(vt4[:, :, 0, 0, :], pv4[:, :, 0, :],
                                           QKV_SCALE, bvb4[:, :, 0, :], mult, add)
            nc.vector.scalar_tensor_tensor(vt4[:, :, 1, 1, :], pv4[:, :, 1, :],
                                           QKV_SCALE, bvb4[:, :, 1, :], mult, add)
            vtq = vt.rearrange("p (a q d) -> p a q d", a=2, q=4)
            nc.vector.tensor_copy(vtq[:, :, 1:3, :], ones_f32[:, 0:256].rearrange(
                "p (a d) -> p a d", a=2).rearrange("p a (u d) -> p a u d", u=2))

        def emit_proj1(b, nch):
            # proj for both ct blocks of one nch half + residual + store
            ns = slice(nch * 512, (nch + 1) * 512)
            o2 = o_sb[b][:].rearrange("p (s n) -> p s n", s=2)
            for ct in range(2):
                pp = ps_f.tile([128, 512], F32, tag="f", name="pp")
                nc.tensor.matmul(pp[:], lhsT=wp2[:, :, ct * 128:(ct + 1) * 128],
                                 rhs=o2[:, :, ns], start=True, stop=True,
                                 perf_mode=DR)
                outt = outpool.tile([128, 512], F32, tag="out")
                nc.vector.scalar_tensor_tensor(outt[:], pp[:], QKV_SCALE,
                                               xb_sb[b][ct][:, ns], mult, add)
                nc.sync.dma_start(y_d[b, ct * 128:(ct + 1) * 128, ns], outt[:])

        # ---------------- attention phase ----------------------------------
        # units per batch in (nch-outer, p-inner) order so proj(nch) can fire
        # after two units; 8 m-steps per unit; S lookahead PIPE=2; filler
        # closures consumed one per step keep the PE queue fed while ACT exps.
        units = [(b, p, nch) for b in range(BL) for nch in range(2) for p in range(2)]
        seq = [(u, m) for u in range(len(units)) for m in range(8)]
        s_tiles = {}
        po_tiles = {}
        ex_pair = {}

        def emit_S(i):
            u, m = seq[i]
            b, p, nch = units[u]
            qt, kt = qk_sb[b][p], qk_sb[b][2 + p]
            ns = slice(nch * 512, (nch + 1) * 512)
            mc = slice(m * 128, (m + 1) * 128)
            ps = ps_s.tile([128, N], F32, tag="s", name="ps")
            nc.tensor.matmul(ps[:, 0:512], lhsT=kt[0:64, mc],
                             rhs=qt[0:64, ns], start=True, stop=True)
            nc.tensor.matmul(ps[:, 512:1024], lhsT=kt[64:128, mc],
                             rhs=qt[64:128, ns], start=True, stop=True)
            s_tiles[i] = ps

        # filler list: (step_idx, closure); consumed inside the main loop
        fillers = []
        fi = [0]

        def run_fillers(k):
            while fi[0] < len(fillers) and fillers[fi[0]][0] <= k:
                fillers[fi[0]][1]()
                fi[0] += 1

        # --------- head: GN b0, first-unit GEMMs, then the main loop -------
        # The PE clock only reaches full p-state after a ~3us CONTINUOUS busy
        # burst, and a slow head leaves every later matmul ~1.4x slower.  The
        # warmup blocks are sized to keep the PE queue gap-free from t~12
        # through the GN matmuls and the DVE chain/h tail into the first qkv,
        # so the ramp completes before the S stream starts.
        emit_gn_stats(0)
        emit_warmups(8)
        emit_gn_chain(0)
        emit_warmups(3)
        emit_h(0)
        # minimal pre-block for unit 0 = (b0, p0, nch0): k01 full + q01 nch0
        # + vt pairs 0-3
        emit_qkv1(0, 2, 0)
        emit_qkv1(0, 2, 1)
        emit_qkv1(0, 0, 0)
        emit_vt1(0, 0)
        emit_vt1(0, 1)
        emit_vt1(0, 2)
        emit_vt1(0, 3)

        # remaining b0 GEMMs as early fillers.  Deadlines (emission order, not
        # just data readiness): S(u,m) is EMITTED at loop iteration u*8+m-PIPE,
        # and a tile written by a filler after that point would get no
        # dependency edge — so every qk/vt write must be emitted strictly
        # before the first S/AV that reads it.
        fillers.append((0, lambda: emit_qkv1(0, 3, 0)))   # k23 n0   (u1 S m0 @ it6)
        fillers.append((1, lambda: emit_qkv1(0, 1, 0)))   # q23 n0   (u1 S m0 @ it6)
        fillers.append((2, lambda: (emit_gn_stats(1), emit_vt1(0, 4))))
        fillers.append((3, lambda: emit_vt1(0, 5)))
        fillers.append((4, lambda: emit_vt1(0, 6)))       # AV u0 j3 @ i7
        fillers.append((5, lambda: (emit_gn_chain(1), emit_vt1(0, 7))))
        fillers.append((6, lambda: emit_qkv1(0, 3, 1)))   # k23 n1   (u1 S m4 @ it10)
        fillers.append((7, lambda: emit_qkv1(0, 0, 1)))   # q01 n1   (u2 S m0 @ it14)
        fillers.append((8, lambda: (emit_h(1), emit_qkv1(0, 1, 1))))  # q23 n1
        # b1 GEMMs; same k-first order
        for k, (ot, nch) in enumerate([(2, 0), (2, 1), (0, 0), (0, 1),
                                       (3, 0), (3, 1), (1, 0), (1, 1)]):
            fillers.append((9 + k, lambda ot=ot, nch=nch: emit_qkv1(1, ot, nch)))
        fillers.append((17, lambda: emit_xb(0)))
        fillers.append((18, lambda: emit_xb(1)))
        for m in range(8):
            fillers.append((19 + m, lambda m=m: emit_vt1(1, m)))
        # proj fillers are appended dynamically (need o tiles) — handled below.

        for b in range(BL):
            o_sb[b] = opool.tile([128, 2 * N], FP8, tag=f"o{b}", name=f"ot{b}")

        PIPE = 2
        for i in range(PIPE):
            emit_S(i)
        for i, (u, m) in enumerate(seq):
            run_fillers(i)
            if i + PIPE < len(seq):
                emit_S(i + PIPE)
            b, p, nch = units[u]
            ns = slice(nch * 512, (nch + 1) * 512)
            h0, h1 = 2 * p, 2 * p + 1
            if m == 0:
                po_tiles[u] = (
                    ps_o.tile([128, 512], F32, tag="o", name="po0"),
                    ps_o.tile([128, 512], F32, tag="o", name="po1"),
                )
            po0, po1 = po_tiles[u]
            ps = s_tiles.pop(i)
            j, slot = divmod(m, 2)
            if slot == 0:
                ex_pair[u] = expool.tile([128, 2 * N], FP8, tag="ex", name="ex")
            ext = ex_pair[u]
            # exp(S - 3): S bounded ~|8| so exp(S-3) <= e^5 fits fp8e4 (max 448)
            # while typical per-column maxima stay in the normal range; the
            # uniform e^-3 scale cancels in the softmax normalization.
            nc.scalar.activation(ext[:, N * slot:N * slot + N], ps[:], Exp,
                                 bias=neg3[:])
            if slot != 1:
                continue
            first, last = (j == 0), (j == 3)
            ex2 = ext[:].rearrange("p (s n) -> p s n", s=2)
            vt2 = vt_sb[b][j][:].rearrange("p (s c) -> p s c", s=2)
            # AV+Z DoubleRow over the m-chunk pair: [V_h0|1] -> O rows 0:64,
            # Zrep rows 64:128; [1|V_h1] mirrored.
            nc.tensor.matmul(
                po0[:], lhsT=vt2[:, :, 128 * h0:128 * h0 + 128],
                rhs=ex2[:, :, 0:512], start=first, stop=last, perf_mode=DR)
            nc.tensor.matmul(
                po1[:], lhsT=vt2[:, :, 128 * h1:128 * h1 + 128],
                rhs=ex2[:, :, 512:1024], start=first, stop=last, perf_mode=DR)
            if not last:
                continue
            # normalize straight out of PSUM; Z rows moved across partitions
            # by a small DMA; output lands in the fp8 o pair tile (slot p).
            ot2 = o_sb[b][:].rearrange("p (s n) -> p s n", s=2)
            # po0: O rows 0:64, Z rows 64:128.  reciprocal_approx_fast is only
            # correct at base partition 0, so copy Z out of PSUM (base-64 copy
            # is fine), DMA-shift down, then recip at base 0.
            zc0 = rzpool.tile([128, 512], F32, tag="zc0", name="zc0")
            nc.vector.tensor_copy(zc0[64:128, :], po0[64:128, :])
            zs0 = rzpool.tile([64, 512], F32, tag="zs0")
            nc.sync.dma_start(zs0[:], zc0[64:128, :])
            # po1: Z rows 0:64 -> recip directly from PSUM at base 0, shift up.
            rz1 = rzpool.tile([128, 512], F32, tag="rz", name="rz1")
            nc.vector.reciprocal_approx_fast(rz1[0:64, :], po1[0:64, :])
            rzs1 = rzpool.tile([128, 512], F32, tag="rzs1")
            nc.sync.dma_start(rzs1[64:128, :], rz1[0:64, :])
            rzs0 = rzpool.tile([64, 512], F32, tag="rzs0")
            nc.vector.reciprocal_approx_fast(rzs0[:], zs0[:])
            nc.vector.tensor_mul(ot2[0:64, p, ns], po0[0:64, :], rzs0[:])
            nc.vector.tensor_mul(ot2[64:128, p, ns], po1[64:128, :], rzs1[64:128, :])
            if DEBUG and u == 0:
                dbgq = ctx.enter_context(tc.tile_pool(name="dbgq", bufs=1))
                dpo0 = dbgq.tile([128, 512], F32, tag="dpo0")
                nc.vector.tensor_copy(dpo0[:], po0[:])
                nc.sync.dma_start(dbg_po_d[0], dpo0[:])
                dpo1 = dbgq.tile([128, 512], F32, tag="dpo1")
                nc.vector.tensor_copy(dpo1[:], po1[:])
                nc.sync.dma_start(dbg_po_d[1], dpo1[:])
                drz = dbgq.tile([128, 512], F32, tag="drz")
                nc.vector.tensor_copy(drz[64:128, :], zc0[64:128, :])
                nc.vector.tensor_copy(drz[0:64, :], rzs0[:])
                nc.sync.dma_start(dbg_rz_d[0], drz[:])
                drz1 = dbgq.tile([128, 512], F32, tag="drz1")
                nc.vector.tensor_copy(drz1[0:64, :], rz1[0:64, :])
                nc.vector.tensor_copy(drz1[64:128, :], rzs1[64:128, :])
                nc.sync.dma_start(dbg_rz_d[1], drz1[:])
            if p == 1:
                # both p-halves of this nch done -> proj can run; emit it as a
                # filler two steps later so it does not delay the next unit's S.
                tgt = min(i + 2, len(seq) - 1)
                fillers.append((tgt, lambda b=b, nch=nch: emit_proj1(b, nch)))
        run_fillers(len(seq))

        if DEBUG:
            dbgpool = ctx.enter_context(tc.tile_pool(name="dbgp", bufs=1))
            for b in range(BL):
                nc.sync.dma_start(dbg_st_d[b], st_sb[b][:])
            for ot in range(4):
                qf = qk_sb[0][ot][:].bitcast(F32)
                nc.sync.dma_start(dbg_qk_d[ot], qf)
            hf = dbgpool.tile([128, 2048], F32, tag="hf")
            nc.vector.tensor_copy(hf[:], h_sb[0][:])
            nc.sync.dma_start(dbg_h_d[:], hf[:])
            vf = dbgpool.tile([128, 1024], F32, tag="vf")
            nc.vector.tensor_copy(vf[:], vt_sb[0][0][:])
            nc.sync.dma_start(dbg_vt_d[:], vf[:])
            of = dbgpool.tile([128, 2048], F32, tag="of")
            nc.vector.tensor_copy(of[:], o_sb[0][:])
            nc.sync.dma_start(dbg_o_d[:], of[:])

    nc.compile()
    return nc


def prep_inputs(x, gn_gamma, gn_beta, qkv_w, qkv_b, proj_w, proj_b):
    """Host-side weight prep shared by kernel() and the test harness."""
    x = np.ascontiguousarray(np.asarray(x, np.float32)).reshape(B, C, N)
    gn_gamma = np.asarray(gn_gamma, np.float32)
    gn_beta = np.asarray(gn_beta, np.float32)
    qkv_w = np.asarray(qkv_w, np.float32)
    qkv_b = np.asarray(qkv_b, np.float32)
    proj_w = np.asarray(proj_w, np.float32)
    proj_b = np.asarray(proj_b, np.float32)

    # fold GroupNorm affine into the qkv GEMM
    W3 = qkv_w * gn_gamma[None, :]
    b3 = qkv_b + qkv_w @ gn_beta
    W3r = W3.reshape(NH, 3, D, C)
    b3r = b3.reshape(NH, 3, D)
    scale = np.float32(D ** -0.5)
    Wq = W3r[:, 0].reshape(C, C)
    Wk = W3r[:, 1].reshape(C, C)          # d^-0.5 folded in the evac constant
    Wv = W3r[:, 2].reshape(C, C)
    bq = b3r[:, 0].reshape(C)
    bk = b3r[:, 1].reshape(C) * scale
    bv = b3r[:, 2].reshape(C)

    def pair_ct(wt):  # [256, out] -> [128, 2*out]: contraction split in 2 slots
        o = wt.shape[1]
        return np.ascontiguousarray(
            wt.reshape(2, 128, o).transpose(1, 0, 2).reshape(128, 2 * o))

    # weights x8 so fp8e4 quantization keeps ~0.5-scale values
    wqk8 = pair_ct((np.concatenate([Wq, Wk], axis=0).T * 8.0).astype(np.float32))
    wv8 = pair_ct((Wv.T * 8.0).astype(np.float32))
    wp8 = pair_ct((proj_w.T * 8.0).astype(np.float32))
    bqk = np.concatenate([bq, bk]).reshape(4, 128)
    bp2 = proj_b.reshape(2, 128)

    cidx = np.arange(128)
    gmap = np.zeros((128, 16), np.float32)
    gmap[cidx, cidx // 8] = 1.0 / 8.0
    gexp = np.zeros((16, 128), np.float32)
    gexp[cidx // 8, cidx] = 1.0

    common = {
        "wqk8": wqk8.astype(np.float32),
        "wv8": wv8.astype(np.float32),
        "wp8": wp8.astype(np.float32),
        "bqk": bqk.astype(np.float32),
        "bv": np.ascontiguousarray(bv[None, :], np.float32),
        "bp2": np.ascontiguousarray(bp2, np.float32),
        "gmap": gmap,
        "gexp": gexp,
    }
    in_maps = [
        {**common, "x": np.ascontiguousarray(x[c * BL:(c + 1) * BL])}
        for c in range(NCORES)
    ]
    return in_maps


_NC_CACHE = []


def kernel(x, gn_gamma, gn_beta, qkv_w, qkv_b, proj_w, proj_b, trace=False):
    in_maps = prep_inputs(x, gn_gamma, gn_beta, qkv_w, qkv_b, proj_w, proj_b)
    if not _NC_CACHE:
        _NC_CACHE.append(build_bass())
    nc = _NC_CACHE[0]
    res = run_bass_kernel_spmd(nc, in_maps, list(range(NCORES)), trace=trace)
    y = np.stack([res.results[c]["y"] for c in range(NCORES)])
    y = y.reshape(B, C, HH, WW)
    kernel.last_result = res
    return y
